# revision 32
# baseline (speedup 1.0000x reference)
"""Multi-head attention (QKV proj + softmax attention + out proj) on 8 TRN2 NeuronCores.

Sharding: batch (2) x head-pairs (4) -> 8 cores. Each core computes q,k,v for its
2 heads of its batch, full attention over the 4096-token sequence for those heads,
and a partial output projection (row-sharded W_proj). The host sums the 4 partial
projections per batch and adds b_proj.

Per-core layout choices:
  - scores are computed transposed ([t, s] = keys on partitions), so the exp'd
    probabilities can feed the AV matmul directly as lhsT with no transposes.
  - the softmax denominator comes for free from a ones-column appended to V
    (row 64 of the [65, s] AV accumulator).
  - the two heads' score matmuls run concurrently on PE row-groups 0-1/2-3
    (K=64 each, lhsT/rhs at base partitions 0 and 64).
  - exp is SPLIT between ScalarE (true exp from PSUM) and VectorE (Schraudolph
    int16 bit-trick: p = bitcast_bf16(i16(round(qk*SCALE*128*log2e + B)))) on a
    strict even/odd key-chunk interleave, so the two engines exponentiate
    concurrently. The Schraudolph constant B is centered (geo-mean preserving)
    so mixing the two exp flavors introduces no systematic softmax bias.
  - score PSUM is a 3-deep ring (6 banks): the next tile's score matmuls fill a
    free slot while both exp engines consume, hiding matmul+semaphore latency.
  - the out-projection of s-chunk sc-2 runs at t=8..11 inside the AV banks'
    idle window (between the AV drain/evac and the next accumulation), so it
    needs no PSUM of its own; the softmax denominators are reshaped [1,512] ->
    [128,8] through a DRAM bounce so the DVE's exact reciprocal (8 cyc/elem
    along the free dim) costs ~130ns instead of 3.3us.
  - ALL qkv projections stream inside the first s-chunk's score loop, packed
    two-projections-per-ring-slot (k+q share one slot, all four v sub-tiles
    share one) so the streaming steals half as many slots from the exps.
"""
from contextlib import ExitStack

import ml_dtypes
import numpy as np

import concourse.bass as bass
import concourse.tile as tile
from concourse import bacc, mybir
from concourse.bass_utils import run_bass_kernel_spmd

B, S, D = 2, 4096, 512
H, HD = 8, 64
SCALE = HD**-0.5
P = 128
SC = 512            # s-chunk width (query positions per inner block)
N_SC = S // SC      # 8
N_T = S // P        # 32 key chunks
KT = D // P         # 4 contraction tiles for the projections
VW = 144            # v_ext row width, 16B-aligned stride: [vA|1A @0..64, pad, vB|1B @72..136]
VB0 = 72            # head-B column offset inside v_ext
BF16 = mybir.dt.bfloat16
F32 = mybir.dt.float32
I16 = mybir.dt.int16
EXP = mybir.ActivationFunctionType.Exp
ALU = mybir.AluOpType

# Schraudolph exp-on-DVE constants: i16 = round(qk_raw * SCHR_A + SCHR_B),
# bitcast to bf16. SCHR_A maps raw-qk units to bf16 exponent LSBs; SCHR_B
# centers so the geometric mean of approx/true exp is 1 (no softmax bias).
SCHR_A = SCALE * 128.0 * np.log2(np.e)     # 23.0831...
SCHR_C = 7.7                               # centering, in i16 LSB units
SCHR_B = 16256.0 - SCHR_C
# odd key-chunks go to the DVE, even ones to ScalarE (strict alternation so
# the two exp engines ping-pong the two score-psum buffers with no same-engine
# serialization)
SCHR_ODD = True

_NC = None


def _emit(tc, out_d, xT_d, wq_d, wk_d, wv_d, bq_d, bk_d, bv_d, wpa_d, reps=1, hw_loop=0):
    nc = tc.nc
    with ExitStack() as ctx:
        consts = ctx.enter_context(tc.tile_pool(name="consts", bufs=1))
        big = ctx.enter_context(tc.tile_pool(name="big", bufs=1))
        xt_pool = ctx.enter_context(tc.tile_pool(name="xt", bufs=1))
        work = ctx.enter_context(tc.tile_pool(name="work", bufs=2))

        wq_sb = consts.tile([P, KT, P], BF16, tag="wq")
        nc.gpsimd.dma_start(wq_sb[:], wq_d.rearrange("(kt p) m -> p kt m", p=P))
        wk_sb = consts.tile([P, KT, P], BF16, tag="wk")
        nc.gpsimd.dma_start(wk_sb[:], wk_d.rearrange("(kt p) m -> p kt m", p=P))
        wv_sb = consts.tile([P, KT, VW], BF16, tag="wv")
        nc.gpsimd.dma_start(wv_sb[:], wv_d.rearrange("(kt p) m -> p kt m", p=P))
        wp_sb = consts.tile([P, D], BF16, tag="wp")
        nc.gpsimd.dma_start(wp_sb[:], wpa_d)
        bq_sb = consts.tile([P, 1], F32, tag="bq")
        nc.gpsimd.dma_start(bq_sb[:], bq_d)
        bk_sb = consts.tile([P, 1], F32, tag="bk")
        nc.gpsimd.dma_start(bk_sb[:], bk_d)
        bv_sb = consts.tile([P, VW], F32, tag="bv")
        nc.gpsimd.dma_start(bv_sb[:], bv_d)
        # row HD (=64) of this is the lhsT of the K=1 broadcast matmul
        ones_sb = consts.tile([P, HD], F32, tag="ones")
        nc.vector.memset(ones_sb[:], 1.0)

        qT_sb = big.tile([P, S], BF16, tag="qT")   # rows 0-63 head A, 64-127 head B
        kT_sb = big.tile([P, S], BF16, tag="kT")
        v_sb = big.tile([P, N_T, VW], BF16, tag="v")
        pAB = big.tile([P, N_T, 2, SC], BF16, tag="p")  # exp'd scoresT per head

        xT_r = xT_d.rearrange("(kt p) s -> p kt s", p=P)

        # PSUM: 3-deep ring of [P, 2, SC] score slots (6 banks) + 2 AV banks.
        # Ring depth 3 hides the score-matmul+semaphore latency behind the two
        # exp engines; qkv (sc=0 only) and the PE warm-up borrow ring slots,
        # while the out-projection borrows the AV banks in their idle window.
        sc_ps = ctx.enter_context(tc.tile_pool(name="scps", bufs=3, space="PSUM"))
        av_ps = ctx.enter_context(tc.tile_pool(name="avps", bufs=1, space="PSUM"))

        def mi_tile(name):
            return sc_ps.tile([P, 2, SC], F32, tag="s", name=name)

        xt_all = xt_pool.tile([P, KT, S], BF16, tag="xtall")

        def load_x(c):
            csl = slice(c * SC, (c + 1) * SC)
            nc.sync.dma_start(xt_all[:, :, csl], xT_r[:, :, csl])
            return xt_all[:, :, csl]

        def kq_chunk(c, xt):
            # k and q projections share ONE ring slot (one [P,512] half each)
            # so sc=0's qkv streaming steals half as many slots from the exps
            csl = slice(c * SC, (c + 1) * SC)
            slot = mi_tile("kq")
            kp, qp = slot[:, 0, :], slot[:, 1, :]
            for kt in range(KT):
                nc.tensor.matmul(kp, lhsT=wk_sb[:, kt, :], rhs=xt[:, kt, :],
                                 start=kt == 0, stop=kt == KT - 1)
            for kt in range(KT):
                nc.tensor.matmul(qp, lhsT=wq_sb[:, kt, :], rhs=xt[:, kt, :],
                                 start=kt == 0, stop=kt == KT - 1)
            nc.scalar.add(out=kT_sb[:, csl], in_=kp, add=bk_sb[:])
            nc.scalar.add(out=qT_sb[:, csl], in_=qp, add=bq_sb[:])

        def v_chunk4(c, xt):
            # all four v sub-tiles ([P,144] each) packed into ONE ring slot
            slot = mi_tile("vv")
            for st in range(4):
                vp = slot[:, st // 2, 256 * (st % 2):256 * (st % 2) + VW]
                for kt in range(KT):
                    nc.tensor.matmul(vp, lhsT=xt[:, kt, st * P:(st + 1) * P],
                                     rhs=wv_sb[:, kt, :],
                                     start=kt == 0, stop=kt == KT - 1)
                nc.vector.tensor_add(out=v_sb[:, c * 4 + st, :], in0=vp, in1=bv_sb[:])

        def scores_t(sc, t):
            ssl = slice(sc * SC, (sc + 1) * SC)
            tsl = slice(t * P, (t + 1) * P)
            sAB = sc_ps.tile([P, 2, SC], F32, tag="s", name="sAB")
            nc.tensor.matmul(sAB[:, 0, :], lhsT=kT_sb[0:HD, tsl],
                             rhs=qT_sb[0:HD, ssl], start=True, stop=True)
            nc.tensor.matmul(sAB[:, 1, :], lhsT=kT_sb[HD:P, tsl],
                             rhs=qT_sb[HD:P, ssl], start=True, stop=True)
            if SCHR_ODD and t % 2 == 1:
                # Schraudolph exp on the DVE: i16(round(qk*A + B)) bitcast bf16
                nc.vector.tensor_scalar(out=pAB[:, t, :, :].bitcast(I16),
                                        in0=sAB[:], scalar1=SCHR_A,
                                        scalar2=SCHR_B, op0=ALU.mult, op1=ALU.add)
            else:
                nc.scalar.activation(out=pAB[:, t, :, :], in_=sAB[:], func=EXP,
                                     scale=SCALE)

        def av_t(avA, avB, t):
            nc.tensor.matmul(avA[0:HD + 1, :], lhsT=v_sb[:, t, 0:HD + 1],
                             rhs=pAB[:, t, 0, :],
                             start=t == 0, stop=t == N_T - 1)
            nc.tensor.matmul(avB[0:HD + 1, :], lhsT=v_sb[:, t, VB0:VB0 + HD + 1],
                             rhs=pAB[:, t, 1, :],
                             start=t == 0, stop=t == N_T - 1)

        def av_evac(avX, h):
            # evacuate the [65, SC] psum accumulator to sbuf so the bank frees
            # early (on ScalarE: the DVE is loaded with Schraudolph tiles)
            avs = work.tile([HD + 1, SC], F32, tag=f"avs{h}")
            nc.scalar.copy(avs[:], avX[0:HD + 1, :])
            return avs

        dr_pool = ctx.enter_context(tc.tile_pool(name="dsc", bufs=2, space="DRAM"))

        def recip_den(avsA, avsB):
            # DVE reciprocal costs 8 cyc/elem along the FREE dim, so fold the
            # two [1, SC] den rows into [128, 8] (DRAM bounce reshape) and the
            # exact reciprocal drops from 2x3.3us to ~130ns. All hops ride the
            # otherwise-idle gpsimd SWDGE queue.
            rd = dr_pool.tile([2, SC], F32, tag="rd")
            nc.gpsimd.dma_start(rd[0:1, :], avsA[HD:HD + 1, :])
            nc.gpsimd.dma_start(rd[1:2, :], avsB[HD:HD + 1, :])
            den_rs = work.tile([P, 8], F32, tag="denrs")
            nc.gpsimd.dma_start(den_rs[:], rd.rearrange("h (p f) -> (h p) f", p=HD))
            rec_rs = work.tile([P, 8], F32, tag="recrs")
            nc.vector.reciprocal(rec_rs[:], den_rs[:])
            rd2 = dr_pool.tile([2, SC], F32, tag="rd2")
            nc.gpsimd.dma_start(rd2.rearrange("h (p f) -> (h p) f", p=HD), rec_rs[:])
            return rd2

        def bcast_rec(rd, h):
            bc = work.tile([HD, SC], F32, tag=f"bc{h}")
            nc.gpsimd.dma_start(bc[:], rd[h:h + 1, :].to_broadcast((HD, SC)))
            return bc

        def norm_head(avs, bc, h, avn=None):
            # avs: [65, SC] sbuf f32. Writes avn rows for head h.
            if avn is None:
                avn = work.tile([P, SC], BF16, tag="avn", name="avn_m")
            if h == 0:
                nc.vector.tensor_mul(avn[0:HD, :], avs[0:HD, :], bc[:])
            else:
                tmp = work.tile([HD, SC], BF16, tag="avnb")
                nc.vector.tensor_mul(tmp[:], avs[0:HD, :], bc[:])
                nc.sync.dma_start(avn[HD:P, :], tmp[:])
            return avn

        def proj_st(sc, avn, st):
            op = av_ps.tile([P, SC], F32, tag=("avA" if st % 2 == 0 else "avB"),
                            name=f"op{st}")
            asl = slice(st * P, (st + 1) * P)
            nc.tensor.matmul(op[:], lhsT=avn[:, asl], rhs=wp_sb[:], start=True,
                             stop=True)
            ot = work.tile([P, D], F32, tag="ot", name=f"ot{st}")
            if st % 2 == 0:
                nc.scalar.copy(ot[:], op[:])
            else:
                nc.vector.tensor_copy(ot[:], op[:])
            r0 = sc * SC + st * P
            nc.sync.dma_start(out_d[r0:r0 + P, :], ot[:])

        def proj_ring(sc, avn, st):
            # epilogue-only: the score ring is drained, use its slots so the
            # 8 tail projections pipeline 3-deep instead of 2-tag serializing
            op = mi_tile(f"opr{st}")[:, 0, :]
            asl = slice(st * P, (st + 1) * P)
            nc.tensor.matmul(op, lhsT=avn[:, asl], rhs=wp_sb[:], start=True,
                             stop=True)
            ot = work.tile([P, D], F32, tag="ot", name=f"otr{st}")
            if st % 2 == 0:
                nc.scalar.copy(ot[:], op)
            else:
                nc.vector.tensor_copy(ot[:], op)
            r0 = sc * SC + st * P
            if st % 2 == 0:
                nc.sync.dma_start(out_d[r0:r0 + P, :], ot[:])
            else:
                nc.gpsimd.dma_start(out_d[r0:r0 + P, :], ot[:])

        LAG = 8  # av group t' = t - LAG runs at score step t

        # PE warm-up burst: junk matmuls that run while the first x DMA is in
        # flight, so the HAM clock-gate is at full rate when real work starts.
        for w in range(10):
            warm = mi_tile("warm")[0:HD, 0, 0:HD]
            nc.tensor.matmul(warm, lhsT=ones_sb[:, 0:HD], rhs=ones_sb[:, 0:HD],
                             start=True, stop=True)

        if hw_loop:
            with tc.For_i(0, hw_loop, 1):
                _emit_body(tc, out_d, xT_r, load_x, kq_chunk, v_chunk4,
                           scores_t, av_t, av_evac, recip_den, bcast_rec,
                           norm_head, proj_st, proj_ring, av_ps, LAG, xt_all)
        else:
            for _rep in range(reps):
                _emit_body(tc, out_d, xT_r, load_x, kq_chunk, v_chunk4,
                           scores_t, av_t, av_evac, recip_den, bcast_rec,
                           norm_head, proj_st, proj_ring, av_ps, LAG, xt_all)


def _emit_body(tc, out_d, xT_r, load_x, kq_chunk, v_chunk4, scores_t,
               av_t, av_evac, recip_den, bcast_rec, norm_head, proj_st,
               proj_ring, av_ps, LAG, xt_all):
        nc = tc.nc
        # prologue: k, q for chunk 0 (v folds into the first loop steps);
        # chunk 0's x comes in per-kt slices so the first k matmul starts at
        # ~1.5us instead of ~6us; chunk 1's x prefetched right away
        for kt in range(KT):
            nc.sync.dma_start(xt_all[:, kt, 0:SC], xT_r[:, kt, 0:SC])
        xt0 = xt_all[:, :, 0:SC]
        xt_nxt = load_x(1)
        kq_chunk(0, xt0)

        prev_av = None      # previous s-chunk's (avA, avB) psum accumulators
        prev_avs = [None, None]
        prev_rec = None
        prev_bc = [None, None]
        prev_avn = None     # avn of chunk sc-1 (normed during sc)
        proj_avn = None     # avn of chunk sc-2 (projected during sc, t=8..11)
        xt_cur = None
        for sc in range(N_SC):
            cur_av = None
            for t in range(N_T):
                # ALL qkv projections stream during sc=0's sweep (q/k/v per
                # chunk; x for chunk c+1 prefetched at the chunk's last slot)
                if sc == 0:
                    if t == 0:
                        v_chunk4(0, xt0)
                    elif t >= 2 and (c := (t - 2) // 4 + 1) <= N_SC - 1:
                        if (t - 2) % 4 == 0:
                            xt_cur = xt_nxt
                            kq_chunk(c, xt_cur)
                        elif (t - 2) % 4 == 2:
                            v_chunk4(c, xt_cur)
                            if c + 1 <= N_SC - 1:
                                xt_nxt = load_x(c + 1)
                if t % 2 == 0:
                    scores_t(sc, t)
                    scores_t(sc, t + 1)
                if prev_av is not None:
                    if t < LAG:
                        av_t(prev_av[0], prev_av[1], N_T - LAG + t)
                        if t == LAG - 1:
                            prev_avs[0] = av_evac(prev_av[0], 0)
                            prev_avs[1] = av_evac(prev_av[1], 1)
                    elif t == LAG:
                        prev_rec = recip_den(prev_avs[0], prev_avs[1])
                    elif t == LAG + 2:
                        prev_bc[0] = bcast_rec(prev_rec, 0)
                        prev_bc[1] = bcast_rec(prev_rec, 1)
                    elif t == LAG + 3:
                        prev_avn = norm_head(prev_avs[0], prev_bc[0], 0)
                    elif t == LAG + 4:
                        norm_head(prev_avs[1], prev_bc[1], 1, prev_avn)
                # project s-chunk sc-2 into the AV banks' idle window t=8..11
                if proj_avn is not None and LAG <= t < LAG + 4:
                    proj_st(sc - 2, proj_avn, t - LAG)
                if t == LAG + 4:
                    avA = av_ps.tile([P, SC], F32, tag="avA", name="avA")
                    avB = av_ps.tile([P, SC], F32, tag="avB", name="avB")
                    cur_av = (avA, avB)
                    for bt in range(t - LAG + 1):   # catch up tiles 0..4
                        av_t(avA, avB, bt)
                elif t > LAG + 4:
                    av_t(cur_av[0], cur_av[1], t - LAG)
            prev_av = cur_av
            proj_avn = prev_avn
            prev_avn = None
        for t in range(LAG):
            av_t(prev_av[0], prev_av[1], N_T - LAG + t)
        avsA = av_evac(prev_av[0], 0)
        avsB = av_evac(prev_av[1], 1)
        rec2 = recip_den(avsA, avsB)
        bcA = bcast_rec(rec2, 0)
        bcB = bcast_rec(rec2, 1)
        for st in range(4):
            proj_ring(N_SC - 2, proj_avn, st)
        avn_m = norm_head(avsA, bcA, 0)
        norm_head(avsB, bcB, 1, avn_m)
        for st in range(4):
            proj_ring(N_SC - 1, avn_m, st)


def build_nc(reps=1, hw_loop=0, **_unused):
    nc = bacc.Bacc("TRN2", target_bir_lowering=False, debug=False, num_devices=8)
    xT = nc.dram_tensor("xT", [D, S], BF16, kind="ExternalInput").ap()
    wq = nc.dram_tensor("wq", [D, P], BF16, kind="ExternalInput").ap()
    wk = nc.dram_tensor("wk", [D, P], BF16, kind="ExternalInput").ap()
    wv = nc.dram_tensor("wv", [D, VW], BF16, kind="ExternalInput").ap()
    bq = nc.dram_tensor("bq", [P, 1], F32, kind="ExternalInput").ap()
    bk = nc.dram_tensor("bk", [P, 1], F32, kind="ExternalInput").ap()
    bv = nc.dram_tensor("bv", [P, VW], F32, kind="ExternalInput").ap()
    wpa = nc.dram_tensor("wp", [P, D], BF16, kind="ExternalInput").ap()
    out = nc.dram_tensor("out", [S, D], F32, kind="ExternalOutput").ap()
    with tile.TileContext(nc) as tc:
        _emit(tc, out, xT, wq, wk, wv, bq, bk, bv, wpa, reps=reps, hw_loop=hw_loop)
    nc.compile()
    return nc


def shard_inputs(x, W_qkv, b_qkv, W_proj):
    bf = ml_dtypes.bfloat16
    xTs = [np.ascontiguousarray(x[b].T).astype(bf) for b in range(B)]
    in_maps = []
    for c in range(8):
        b, hp = divmod(c, 4)
        h0 = 2 * hp
        qc = slice(h0 * HD, h0 * HD + P)
        kc = slice(D + h0 * HD, D + h0 * HD + P)
        v0 = 2 * D + h0 * HD
        wv = np.zeros((D, VW), np.float32)
        wv[:, 0:HD] = W_qkv[:, v0:v0 + HD]
        wv[:, VB0:VB0 + HD] = W_qkv[:, v0 + HD:v0 + 2 * HD]
        bv = np.zeros((VW,), np.float32)
        bv[0:HD] = b_qkv[v0:v0 + HD]
        bv[HD] = 1.0
        bv[VB0:VB0 + HD] = b_qkv[v0 + HD:v0 + 2 * HD]
        bv[VB0 + HD] = 1.0
        in_maps.append({
            "xT": xTs[b],
            "wq": np.ascontiguousarray(W_qkv[:, qc]).astype(bf),
            "wk": np.ascontiguousarray(W_qkv[:, kc]).astype(bf),
            "wv": wv.astype(bf),
            "bq": np.ascontiguousarray(b_qkv[qc]).reshape(P, 1).astype(np.float32),
            "bk": np.ascontiguousarray(b_qkv[kc]).reshape(P, 1).astype(np.float32),
            "bv": np.tile(bv[None, :], (P, 1)).astype(np.float32),
            "wp": np.ascontiguousarray(W_proj[hp * P:(hp + 1) * P, :]).astype(bf),
        })
    return in_maps


def kernel(x, W_qkv, b_qkv, W_proj, b_proj):
    x = np.asarray(x, np.float32)
    W_qkv = np.asarray(W_qkv, np.float32)
    b_qkv = np.asarray(b_qkv, np.float32)
    W_proj = np.asarray(W_proj, np.float32)
    b_proj = np.asarray(b_proj, np.float32)

    global _NC
    if _NC is None:
        _NC = build_nc()
    in_maps = shard_inputs(x, W_qkv, b_qkv, W_proj)
    res = run_bass_kernel_spmd(_NC, in_maps, core_ids=list(range(8)))
    outs = [r["out"].astype(np.float32) for r in res.results]
    full = np.stack([outs[4 * b] + outs[4 * b + 1] + outs[4 * b + 2] + outs[4 * b + 3] + b_proj
                     for b in range(B)])
    return full.astype(np.float32)
